# revision 12
# baseline (speedup 1.0000x reference)
"""CoAttentionLayer3 kernel: data-parallel over batch across 8 NeuronCores.

Device (bass/Tile) computes the dominant FLOPs: the two to_dim projections
d = LN(x) @ Wd.T for the batch shard, as d^T = WdT.T-chunks @ xT with the
contraction dim (512) on partitions. LN + the tiny attention tail
(att 16x16 per (b,h), joint softmax, out1 einsums, final 1024->2 proj)
run on host: together they are <5% of FLOPs and shape-hostile to the PE.

Inputs are pre-transposed on host so the kernel needs zero on-device
transposes: xT (512 dim, 512 rows=(32b x 16n)), WdT (512 dim, 1024 inner).
"""

import numpy as np

B, N, DIM = 256, 16, 512
HEADS, DHEAD = 16, 64
INNER = HEADS * DHEAD
EPS = 1e-5
NCORES = 8
BS = B // NCORES          # 32 batches per core
ROWS = BS * N             # 512 rows per core
P = 128


def _build_nc():
    import concourse.bass as bass
    from concourse import mybir

    nc = bass.Bass()
    f32 = mybir.dt.float32

    x1T = nc.declare_dram_parameter("x1T", [DIM, ROWS], f32, isOutput=False)
    x2T = nc.declare_dram_parameter("x2T", [DIM, ROWS], f32, isOutput=False)
    wdT = nc.declare_dram_parameter("wdT", [DIM, INNER], f32, isOutput=False)
    d1T = nc.declare_dram_parameter("d1T", [INNER, ROWS], f32, isOutput=True)
    d2T = nc.declare_dram_parameter("d2T", [INNER, ROWS], f32, isOutput=True)

    KT = DIM // P      # 4 contraction tiles
    NC_CH = INNER // P  # 8 inner chunks per drug
    from contextlib import ExitStack
    ctx = ExitStack()
    wt = [ctx.enter_context(nc.sbuf_tensor(f'wt{k}', [P, INNER], f32)) for k in range(KT)]
    xt = [[ctx.enter_context(nc.sbuf_tensor(f'xt{d}_{k}', [P, ROWS], f32)) for k in range(KT)]
          for d in range(2)]
    ot = [ctx.enter_context(nc.sbuf_tensor(f'ot{i}', [P, ROWS], f32)) for i in range(4)]
    ps = [ctx.enter_context(nc.psum_tensor(f'ps{i}', [P, ROWS], f32)) for i in range(8)]

    with ctx, nc.Block() as block, \
            nc.semaphore("in_sem") as in_sem, \
            nc.semaphore("mm_sem") as mm_sem, \
            nc.semaphore("cp_sem") as cp_sem, \
            nc.semaphore("out_sem") as out_sem:

        @block.gpsimd
        def _(g):
            for k in range(KT):
                g.dma_start(out=wt[k][:], in_=wdT[k * P:(k + 1) * P, :]).then_inc(in_sem, 16)
            for d in range(2):
                src_ = (x1T, x2T)[d]
                for k in range(KT):
                    g.dma_start(out=xt[d][k][:], in_=src_[k * P:(k + 1) * P, :]).then_inc(in_sem, 16)
            for gi in range(16):
                d, c = divmod(gi, NC_CH)
                g.wait_ge(cp_sem, gi + 1)
                g.dma_start(out=(d1T, d2T)[d][c * P:(c + 1) * P, :],
                            in_=ot[gi % 4][:]).then_inc(out_sem, 16)
            g.wait_ge(out_sem, 16 * 16)

        @block.tensor
        def _(t):
            for gi in range(16):
                d, c = divmod(gi, NC_CH)
                t.wait_ge(in_sem, 16 * (4 + 4 * d + KT))
                if gi >= 8:
                    t.wait_ge(cp_sem, gi - 7)
                for k in range(KT):
                    mm = t.matmul(ps[gi % 8][:], wt[k][:, c * P:(c + 1) * P],
                                  xt[d][k][:], start=(k == 0), stop=(k == KT - 1))
                mm.then_inc(mm_sem, 1)

        @block.vector
        def _(v):
            for gi in range(16):
                v.wait_ge(mm_sem, gi + 1)
                if gi >= 4:
                    v.wait_ge(out_sem, 16 * (gi - 3))
                v.tensor_copy(out=ot[gi % 4][:], in_=ps[gi % 8][:]).then_inc(cp_sem, 1)
    return nc


_NC_CACHE = None
LAST_EXEC_NS = None


def kernel(drug1, drug2, ln_w, ln_b, Wd, Wout, bout):
    from concourse.bass_utils import run_bass_kernel_spmd

    global _NC_CACHE
    if _NC_CACHE is None:
        _NC_CACHE = _build_nc()
    nc = _NC_CACHE

    drug1 = np.asarray(drug1, np.float32)
    drug2 = np.asarray(drug2, np.float32)
    ln_w = np.asarray(ln_w, np.float32)
    ln_b = np.asarray(ln_b, np.float32)
    Wd = np.asarray(Wd, np.float32)
    Wout = np.asarray(Wout, np.float32)
    bout = np.asarray(bout, np.float32)

    def ln(x):
        mu = x.mean(-1, keepdims=True)
        var = ((x - mu) ** 2).mean(-1, keepdims=True)
        return (x - mu) / np.sqrt(var + EPS) * ln_w + ln_b

    x1 = ln(drug1).reshape(B * N, DIM)
    x2 = ln(drug2).reshape(B * N, DIM)
    wdT = np.ascontiguousarray(Wd.T)  # (512, 1024)

    in_maps = []
    for c in range(NCORES):
        sl = slice(c * ROWS, (c + 1) * ROWS)
        in_maps.append({
            "x1T": np.ascontiguousarray(x1[sl].T),
            "x2T": np.ascontiguousarray(x2[sl].T),
            "wdT": wdT,
        })

    import time as _t
    _t0 = _t.time()
    res = run_bass_kernel_spmd(nc, in_maps, core_ids=list(range(NCORES)))
    global LAST_EXEC_NS
    LAST_EXEC_NS = res.exec_time_ns or int((_t.time() - _t0) * 1e9)
    if res.exec_time_ns:
        print(f"HW exec time: {res.exec_time_ns} ns")

    # Gather: d^T (1024, 512) per core -> d (b, h, n, dd)
    d1 = np.empty((B, HEADS, N, DHEAD), np.float32)
    d2 = np.empty((B, HEADS, N, DHEAD), np.float32)
    for c in range(NCORES):
        bsl = slice(c * BS, (c + 1) * BS)
        for arr, key in ((d1, "d1T"), (d2, "d2T")):
            dT = res.results[c][key]  # (INNER, ROWS)
            # rows = (b, n), inner = (h, dd)
            arr[bsl] = (
                dT.T.reshape(BS, N, HEADS, DHEAD).transpose(0, 2, 1, 3)
            )

    scale = DHEAD ** -0.5
    att = np.einsum("bhnd,bhmd->bhnm", d1, d2) * scale
    flat = att.reshape(B, HEADS, N * N)
    flat = flat - flat.max(-1, keepdims=True)
    e = np.exp(flat)
    A = (e / e.sum(-1, keepdims=True)).reshape(B, HEADS, N, N)
    out1 = (np.einsum("biqk,bqkd->bid", A, d1)
            + np.einsum("biqk,bikd->bid", A, d2))
    out2 = out1.reshape(B, HEADS * DHEAD)
    return (out2 @ Wout.T + bout).astype(np.float32)
